# revision 19
# baseline (speedup 1.0000x reference)
"""Trainium2 Bass kernel for nn_CrossAttention (single-head NxN attention + proj + InstanceNorm + residual).

v4: everything from v3 (fused just-in-time k/v production, per-i-tile
stats AllGather, single-HAM-window PE stream) plus:
 - PE warmup dummy matmuls during the DMA-gated start (HAM reaches
   2.4 GHz before real work arrives)
 - host-fused projection: wpv = wp @ wv, so the V matmul directly
   produces projected values and the per-tile proj matmuls disappear
 - fp16 residual (drops the 2MB fp32 x1 load)
 - 1024-wide final normalize chunks with 4-deep buffers

Sharding: 8 cores = (batch b in 0..3) x (query-half h in 0..1).
Self-contained: hardcodes shapes B=4, C=256, D=H=W=16 (N=4096), Cr=32.
"""

import numpy as np

import concourse.bass as bass
import concourse.mybir as mybir
import concourse.tile as tile
from concourse import bacc
from concourse.bass_utils import run_bass_kernel_spmd
from concourse.masks import make_identity

B, C, N, Cr = 4, 256, 4096, 32
NH = N // 2  # query tokens per core
EPS = 1e-5
SCALE = float(Cr) ** -0.5
FP32 = mybir.dt.float32
FP16 = mybir.dt.float16

N_CORES = 8
REPLICA_GROUPS = [[0, 1], [2, 3], [4, 5], [6, 7]]

IT = 512                   # i-tile width (query columns processed together)
N_ITILES = NH // IT        # 4
JBLK = 128                 # j-block (rows per QK matmul output)
N_JBLK = N // JBLK         # 32
JB_PER_BURST = 2           # j-blocks per burst; each row-tiled QK matmul owns a full PSUM bank
N_JBURSTS = N_JBLK // JB_PER_BURST  # 16

AF = mybir.ActivationFunctionType
ALU = mybir.AluOpType

LAST_RESULTS = None  # BassKernelResults of the most recent run (for test harness)


def build_nc(use_collective=True):
    nc = bacc.Bacc("TRN2", num_devices=N_CORES, name="xattn",
                   target_bir_lowering=False)

    x1f_d = nc.dram_tensor("x1f", [C, NH], FP16, kind="ExternalInput").ap()
    x2b_d = nc.dram_tensor("x2b", [C, N], FP16, kind="ExternalInput").ap()
    wq4_d = nc.dram_tensor("wq4", [C, 128], FP16, kind="ExternalInput").ap()
    wk4_d = nc.dram_tensor("wk4", [C, 128], FP16, kind="ExternalInput").ap()
    wpvT_d = nc.dram_tensor("wpvT", [C, C], FP16, kind="ExternalInput").ap()
    out_d = nc.dram_tensor("out", [C, NH], FP32, kind="ExternalOutput").ap()

    with tile.TileContext(nc) as tc:
        build_body(tc, x1f_d, x2b_d, wq4_d, wk4_d, wpvT_d, out_d,
                   use_collective)
    nc.compile()
    return nc


def build_body(tc, x1f_d, x2b_d, wq4_d, wk4_d, wpvT_d, out_d,
               use_collective=True):
    nc = tc.nc
    from contextlib import ExitStack

    with ExitStack() as ctx:
        persist = ctx.enter_context(tc.tile_pool(name="persist", bufs=1))
        ptp = ctx.enter_context(tc.tile_pool(name="ptp", bufs=3))
        sm = ctx.enter_context(tc.tile_pool(name="sm", bufs=4))
        sm2 = ctx.enter_context(tc.tile_pool(name="sm2", bufs=2))
        sm4 = ctx.enter_context(tc.tile_pool(name="sm4", bufs=4))
        qkp = ctx.enter_context(tc.tile_pool(name="qkp", bufs=2, space="PSUM"))
        avp = ctx.enter_context(tc.tile_pool(name="avp", bufs=4, space="PSUM"))
        dramp = ctx.enter_context(tc.tile_pool(name="dramp", bufs=1, space="DRAM"))

        # ---- PE warmup: dense dummy matmuls while DMA streams in -------
        # HAM un-throttles after ~3.4us of sustained PE activity; these
        # run during the input-DMA window so real work starts at 2.4 GHz.
        # The result feeds the warmup collective's input so DCE keeps it.
        scr = persist.tile([128, 64], FP16, tag="scr", name="scr")
        nc.vector.memset(scr, 0.5)
        wup = qkp.tile([128, 64], FP32, tag="qk", name="wup")
        for w in range(64):
            nc.tensor.matmul(wup[0:64, :], lhsT=scr, rhs=scr,
                             start=(w == 0), stop=(w == 63))
        wup_sb = persist.tile([128, 4], FP32, tag="wup_sb", name="wup_sb")
        nc.vector.tensor_copy(wup_sb[0:64, :], wup[0:64, 0:4])
        nc.vector.memset(wup_sb[64:128, :], 0.0)

        # ---- warmup collective: absorb CC stream startup ----------------
        if use_collective:
            warm_in = dramp.tile([128, 4], FP32, tag="warm_i", name="warm_in")
            warm_out = dramp.tile([2, 128, 4], FP32, tag="warm_o", name="warm_out")
            nc.sync.dma_start(warm_in, wup_sb)
            nc.gpsimd.collective_compute(
                "AllGather", ALU.bypass, replica_groups=REPLICA_GROUPS,
                ins=[warm_in.opt()], outs=[warm_out.opt()])

        # ---- constants -------------------------------------------------
        eps_sb = persist.tile([128, 1], FP32, tag="eps", name="eps_sb")
        nc.vector.memset(eps_sb, EPS)
        ident = persist.tile([128, 128], FP32, tag="ident", name="ident")
        make_identity(nc, ident)
        ident_hf = persist.tile([128, 128], FP16, tag="identh", name="ident_hf")
        nc.vector.tensor_copy(ident_hf, ident)

        # ---- input loads, ordered by first use -------------------------
        wq_sb, wk_sb, wpv_sb = [], [], []
        for cc in range(2):
            w1 = persist.tile([128, 128], FP16, tag=f"wq{cc}", name=f"wq_sb{cc}")
            nc.sync.dma_start(w1, wq4_d[128 * cc:128 * (cc + 1), :])
            wq_sb.append(w1)
            w2 = persist.tile([128, 128], FP16, tag=f"wk{cc}", name=f"wk_sb{cc}")
            nc.sync.dma_start(w2, wk4_d[128 * cc:128 * (cc + 1), :])
            wk_sb.append(w2)
        x1_hf = [persist.tile([128, NH], FP16, tag=f"x1f_{cc}", name=f"x1_hf{cc}")
                 for cc in range(2)]
        x2_sb = [persist.tile([128, N], FP16, tag=f"x2_{cc}", name=f"x2_sb{cc}")
                 for cc in range(2)]

        # 1024-col chunks (2KB per partition line) amortize DMA packet
        # overhead; later chunks are emitted just-in-time from the main
        # loop so early bandwidth goes to the first-needed data.
        def dma_x2(chunk):
            sl = slice(1024 * chunk, 1024 * (chunk + 1))
            for cc in range(2):
                nc.sync.dma_start(x2_sb[cc][:, sl],
                                  x2b_d[128 * cc:128 * (cc + 1), sl])

        def dma_x1f(chunk):
            sl = slice(1024 * chunk, 1024 * (chunk + 1))
            for cc in range(2):
                nc.sync.dma_start(x1_hf[cc][:, sl],
                                  x1f_d[128 * cc:128 * (cc + 1), sl])

        dma_x1f(0)
        dma_x2(0)
        for cc in range(2):
            w3 = persist.tile([128, C], FP16, tag=f"wpv{cc}", name=f"wpv_sb{cc}")
            nc.sync.dma_start(w3, wpvT_d[128 * cc:128 * (cc + 1), :])
            wpv_sb.append(w3)
        dma_x2(1)

        # ---- persistent main-loop operands -----------------------------
        vt = persist.tile([128, N_JBLK, C + 1], FP16, tag="vt", name="vt")
        nc.vector.memset(vt[:, :, C:C + 1], 1.0)
        k_rep = persist.tile([128, N], FP16, tag="krep", name="k_rep")
        q_rep = persist.tile([128, NH], FP16, tag="qrep", name="q_rep")

        def emit_q(qt):
            qp = qkp.tile([128, 512], FP32, tag="qk", name=f"qp{qt}")
            for cc in range(2):
                nc.tensor.matmul(
                    qp, lhsT=wq_sb[cc],
                    rhs=x1_hf[cc][:, 512 * qt:512 * (qt + 1)],
                    start=(cc == 0), stop=(cc == 1))
            nc.vector.tensor_copy(q_rep[:, 512 * qt:512 * (qt + 1)], qp)

        def emit_k(jt):
            kp = qkp.tile([128, 512], FP32, tag="qk", name=f"kp{jt}")
            for cc in range(2):
                nc.tensor.matmul(
                    kp, lhsT=wk_sb[cc],
                    rhs=x2_sb[cc][:, 512 * jt:512 * (jt + 1)],
                    start=(cc == 0), stop=(cc == 1))
            nc.vector.tensor_copy(k_rep[:, 512 * jt:512 * (jt + 1)], kp)

        def emit_v(jblk):
            vp = qkp.tile([128, C], FP32, tag="qk", name=f"vp{jblk}")
            for cc in range(2):
                nc.tensor.matmul(
                    vp, lhsT=x2_sb[cc][:, 128 * jblk:128 * (jblk + 1)],
                    rhs=wpv_sb[cc], start=(cc == 0), stop=(cc == 1))
            nc.vector.tensor_copy(vt[:, jblk, 0:C], vp)

        def produce_for_burst(b):
            # emit k-chunk / v-blocks needed by tile-0 burst b
            if b % 2 == 0 and b // 2 >= 1:
                emit_k(b // 2)
            emit_v(2 * b)
            emit_v(2 * b + 1)

        # minimal pre-main production: q/k/v for the first two bursts;
        # everything else is produced just-in-time inside tile 0
        emit_q(0)
        emit_k(0)
        for b in range(2):
            emit_v(2 * b)
            emit_v(2 * b + 1)

        # ---- per-i-tile stats + AllGather state ------------------------
        proj_sb = [persist.tile([128, NH], FP16, tag=f"proj{ob}", name=f"proj_sb{ob}")
                   for ob in range(2)]
        st_sb = [persist.tile([128, 2, 6], FP32, tag=f"st{it}", name=f"st{it}")
                 for it in range(N_ITILES)]
        # gathered stats: [p, tile, rank, ob, 6]
        cc_all = persist.tile([128, N_ITILES, 2, 2, 6], FP32, tag="cc_all",
                              name="cc_all")
        st_dr = [dramp.tile([128, 12], FP32, tag=f"sti{it}", name=f"st_dr{it}")
                 for it in range(N_ITILES)]
        ag_dr = [dramp.tile([2, 128, 12], FP32, tag=f"sto{it}", name=f"ag_dr{it}")
                 for it in range(N_ITILES)]

        def emit_stats_ag(it):
            nc.sync.dma_start(st_dr[it], st_sb[it].rearrange("p o s -> p (o s)"))
            if use_collective:
                nc.gpsimd.collective_compute(
                    "AllGather", ALU.bypass, replica_groups=REPLICA_GROUPS,
                    ins=[st_dr[it].opt()], outs=[ag_dr[it].opt()])
                nc.sync.dma_start(
                    cc_all[:, it],
                    ag_dr[it].rearrange("r p c -> p r c")
                             .rearrange("p r (o s) -> p r o s", o=2))
            else:
                nc.vector.tensor_copy(cc_all[:, it, 0], st_sb[it])
                nc.vector.tensor_copy(cc_all[:, it, 1], st_sb[it])

        def emit_qk(it, jb):
            isl = slice(IT * it, IT * (it + 1))
            qk = qkp.tile([128, IT * JB_PER_BURST], FP32, tag="qk",
                          name=f"qk{it}_{jb}")
            for t in range(JB_PER_BURST):
                jblk = jb * JB_PER_BURST + t
                rt = t + 2 * (jb % 2)   # alternate row-groups between bursts
                nc.tensor.matmul(
                    qk[:, IT * t:IT * (t + 1)],
                    lhsT=k_rep[32 * rt:32 * (rt + 1),
                               JBLK * jblk:JBLK * (jblk + 1)],
                    rhs=q_rep[32 * rt:32 * (rt + 1), isl],
                    start=True, stop=True, tile_position=(32 * rt, 0))
            pt = ptp.tile([128, IT * JB_PER_BURST], FP16, tag="pt",
                          name=f"pt{it}_{jb}")
            nc.scalar.activation(out=pt, in_=qk, func=AF.Exp)
            return pt

        def emit_av(av_t, jb, pt):
            for t in range(JB_PER_BURST):
                jblk = jb * JB_PER_BURST + t
                for ib in range(4):
                    nc.tensor.matmul(
                        av_t[ib],
                        lhsT=pt[:, IT * t + 128 * ib:IT * t + 128 * (ib + 1)],
                        rhs=vt[:, jblk, :],
                        start=(jb == 0 and t == 0),
                        stop=(jb == N_JBURSTS - 1 and t == JB_PER_BURST - 1))

        def epilogue_part1(it, av_t):
            # normalize by softmax denominator, transpose [i,o] -> [o,i]
            # straight into proj_sb. transposes allocate from the qk pool
            # (tag "qk") so next-tile QK interleaves in the same PSUM slots.
            isl0 = IT * it
            for ib in range(4):
                rden = sm.tile([128, 1], FP32, tag="rden", name=f"rden{it}_{ib}")
                nc.vector.reciprocal(rden, av_t[ib][:, C:C + 1])
                avn = sm.tile([128, C], FP16, tag="avn", name=f"avn{it}_{ib}")
                nc.vector.tensor_scalar_mul(avn, in0=av_t[ib][:, 0:C], scalar1=rden)
                tp = qkp.tile([128, C], FP16, tag="qk", name=f"tp{it}_{ib}")
                nc.tensor.transpose(tp[:, 0:128], avn[:, 0:128], ident_hf)
                nc.tensor.transpose(tp[:, 128:256], avn[:, 128:256], ident_hf)
                for ob in range(2):
                    nc.vector.tensor_copy(
                        proj_sb[ob][:, isl0 + 128 * ib:isl0 + 128 * (ib + 1)],
                        tp[:, 128 * ob:128 * (ob + 1)])

        def epilogue_part2(it):
            isl = slice(IT * it, IT * (it + 1))
            for ob in range(2):
                nc.vector.bn_stats(st_sb[it][:, ob, :], proj_sb[ob][:, isl])
            emit_stats_ag(it)

        # ---- main attention loop, software-pipelined across bursts -----
        pt_hold = emit_qk(0, 0)
        pending1 = None   # (it, av_t) awaiting epilogue part 1
        pending2 = None   # it awaiting epilogue part 2
        for it in range(N_ITILES):
            av_t = [avp.tile([128, C + 1], FP32, tag="av", name=f"av{it}_{ib}")
                    for ib in range(4)]
            for jb in range(N_JBURSTS):
                last = (it == N_ITILES - 1 and jb == N_JBURSTS - 1)
                if not last:
                    nit, njb = (it, jb + 1) if jb + 1 < N_JBURSTS else (it + 1, 0)
                    pt_next = emit_qk(nit, njb)
                else:
                    pt_next = None
                emit_av(av_t, jb, pt_hold)
                pt_hold = pt_next
                if it == 0 and jb in (0, 4):
                    dma_x2(2 + jb // 4)
                if it == 0 and jb == 8:
                    dma_x1f(1)
                if it == 0 and jb <= 13:
                    produce_for_burst(jb + 2)
                if jb == 14 and it + 1 < N_ITILES:
                    emit_q(it + 1)
                if jb == 0 and pending1 is not None:
                    epilogue_part1(*pending1)
                    pending2 = pending1[0]
                    pending1 = None
                if jb == 1 and pending2 is not None:
                    epilogue_part2(pending2)
                    pending2 = None
            pending1 = (it, av_t)
        epilogue_part1(*pending1)
        epilogue_part2(N_ITILES - 1)

        # ---- final stats combine (needs all 4 gathered tiles) ----------
        mean_var = persist.tile([128, 2, 2], FP32, tag="mv2", name="mean_var")
        for ob in range(2):
            nc.vector.bn_aggr(out=mean_var[:, ob, :],
                              in_=cc_all.rearrange("p t r o s -> p (t r) o s")
                                        [:, :, ob, :])
        std2 = sm.tile([128, 2], FP32, tag="std2", name="std2")
        # sqrt's ACT table load slots in right after the last exp, hiding
        # under the collective wait
        nc.scalar.activation(out=std2, in_=mean_var[:, :, 1], func=AF.Sqrt,
                             bias=eps_sb, scale=1.0)
        rstd2 = persist.tile([128, 2], FP32, tag="rstd2", name="rstd2")
        nc.vector.reciprocal(rstd2, std2)
        # nmr = -mean * rstd  (per-partition bias for the ACT normalize)
        nmr2 = persist.tile([128, 2], FP32, tag="nmr2", name="nmr2")
        nc.vector.tensor_mul(nmr2, mean_var[:, :, 0], rstd2)
        nc.vector.tensor_scalar(out=nmr2, in0=nmr2, scalar1=-1.0, scalar2=None,
                                op0=ALU.mult)

        for ob in range(2):
            rstd = rstd2[:, ob:ob + 1]
            nmr = nmr2[:, ob:ob + 1]
            # normed = proj*rstd - mean*rstd on ACT; residual add on DVE;
            # 1024-wide chunks, 4-deep buffers so nothing stalls on WAR
            for ch in range(2):
                sl = slice(1024 * ch, 1024 * (ch + 1))
                nrm = sm4.tile([128, 1024], FP16, tag="nrm", name=f"nrm{ob}_{ch}")
                nc.scalar.activation(out=nrm, in_=proj_sb[ob][:, sl],
                                     func=AF.Identity, bias=nmr, scale=rstd)
                ot = sm4.tile([128, 1024], FP32, tag="ot", name=f"ot{ob}_{ch}")
                nc.vector.tensor_add(ot, nrm, x1_hf[ob][:, sl])
                nc.sync.dma_start(out_d[128 * ob:128 * (ob + 1), sl], ot)


_nc_cache = None


def _get_nc():
    global _nc_cache
    if _nc_cache is None:
        _nc_cache = build_nc()
    return _nc_cache


def make_in_maps(x1, x2, wq, wk, wv, wp):
    x1f = np.ascontiguousarray(x1, dtype=np.float32).reshape(B, C, N)
    x1h16 = x1f.astype(np.float16)
    x2f = np.asarray(x2, np.float32).reshape(B, C, N).astype(np.float16)
    wqT = ((np.asarray(wq, np.float32) * SCALE).T).astype(np.float16)
    wkT = (np.asarray(wk, np.float32).T).astype(np.float16)
    wq4 = np.ascontiguousarray(np.tile(wqT, (1, 4)))   # [C, 128]
    wk4 = np.ascontiguousarray(np.tile(wkT, (1, 4)))   # [C, 128]
    # fused projection: proj(v) = (wp @ wv) @ x2
    wpv = np.asarray(wp, np.float32) @ np.asarray(wv, np.float32)
    wpvT = np.ascontiguousarray(wpv.T.astype(np.float16))
    in_maps = []
    for core in range(N_CORES):
        b, h = core // 2, core % 2
        in_maps.append({
            "x1f": np.ascontiguousarray(x1h16[b, :, h * NH:(h + 1) * NH]),
            "x2b": np.ascontiguousarray(x2f[b]),
            "wq4": wq4, "wk4": wk4, "wpvT": wpvT,
        })
    return in_maps


def assemble_out(results):
    out = np.empty((B, C, N), np.float32)
    for core in range(N_CORES):
        b, h = core // 2, core % 2
        out[b, :, h * NH:(h + 1) * NH] = results[core]["out"]
    return out.reshape(B, C, 16, 16, 16)


def kernel(**inputs):
    global LAST_RESULTS
    in_maps = make_in_maps(inputs["x1"], inputs["x2"], inputs["wq"],
                           inputs["wk"], inputs["wv"], inputs["wp"])
    res = run_bass_kernel_spmd(_get_nc(), in_maps, core_ids=list(range(N_CORES)))
    LAST_RESULTS = res
    return assemble_out(res.results)


# revision 21
# speedup vs baseline: 1.0442x; 1.0442x over previous
"""Trainium2 Bass kernel for nn_CrossAttention (single-head NxN attention + proj + InstanceNorm + residual).

v4: everything from v3 (fused just-in-time k/v production, per-i-tile
stats AllGather, single-HAM-window PE stream) plus:
 - PE warmup dummy matmuls during the DMA-gated start (HAM reaches
   2.4 GHz before real work arrives)
 - host-fused projection: wpv = wp @ wv, so the V matmul directly
   produces projected values and the per-tile proj matmuls disappear
 - fp16 residual (drops the 2MB fp32 x1 load)
 - 1024-wide final normalize chunks with 4-deep buffers

Sharding: 8 cores = (batch b in 0..3) x (query-half h in 0..1).
Self-contained: hardcodes shapes B=4, C=256, D=H=W=16 (N=4096), Cr=32.
"""

import numpy as np

import concourse.bass as bass
import concourse.mybir as mybir
import concourse.tile as tile
from concourse import bacc
from concourse.bass_utils import run_bass_kernel_spmd
from concourse.masks import make_identity

B, C, N, Cr = 4, 256, 4096, 32
NH = N // 2  # query tokens per core
EPS = 1e-5
SCALE = float(Cr) ** -0.5
FP32 = mybir.dt.float32
FP16 = mybir.dt.float16

N_CORES = 8
REPLICA_GROUPS = [[0, 1], [2, 3], [4, 5], [6, 7]]

IT = 512                   # i-tile width (query columns processed together)
N_ITILES = NH // IT        # 4
JBLK = 128                 # j-block (rows per QK matmul output)
N_JBLK = N // JBLK         # 32
JB_PER_BURST = 2           # j-blocks per burst; each row-tiled QK matmul owns a full PSUM bank
N_JBURSTS = N_JBLK // JB_PER_BURST  # 16

AF = mybir.ActivationFunctionType
ALU = mybir.AluOpType

LAST_RESULTS = None  # BassKernelResults of the most recent run (for test harness)


def build_nc(use_collective=True):
    nc = bacc.Bacc("TRN2", num_devices=N_CORES, name="xattn",
                   target_bir_lowering=False)

    x1f_d = nc.dram_tensor("x1f", [C, NH], FP16, kind="ExternalInput").ap()
    x2b_d = nc.dram_tensor("x2b", [C, N], FP16, kind="ExternalInput").ap()
    wq4_d = nc.dram_tensor("wq4", [C, 128], FP16, kind="ExternalInput").ap()
    wk4_d = nc.dram_tensor("wk4", [C, 128], FP16, kind="ExternalInput").ap()
    wpvT_d = nc.dram_tensor("wpvT", [C, C], FP16, kind="ExternalInput").ap()
    out_d = nc.dram_tensor("out", [C, NH], FP32, kind="ExternalOutput").ap()

    with tile.TileContext(nc) as tc:
        build_body(tc, x1f_d, x2b_d, wq4_d, wk4_d, wpvT_d, out_d,
                   use_collective)
    nc.compile()
    return nc


def build_body(tc, x1f_d, x2b_d, wq4_d, wk4_d, wpvT_d, out_d,
               use_collective=True):
    nc = tc.nc
    from contextlib import ExitStack

    with ExitStack() as ctx:
        persist = ctx.enter_context(tc.tile_pool(name="persist", bufs=1))
        ptp = ctx.enter_context(tc.tile_pool(name="ptp", bufs=3))
        sm = ctx.enter_context(tc.tile_pool(name="sm", bufs=4))
        sm2 = ctx.enter_context(tc.tile_pool(name="sm2", bufs=2))
        sm4 = ctx.enter_context(tc.tile_pool(name="sm4", bufs=4))
        qkp = ctx.enter_context(tc.tile_pool(name="qkp", bufs=2, space="PSUM"))
        avp = ctx.enter_context(tc.tile_pool(name="avp", bufs=4, space="PSUM"))
        dramp = ctx.enter_context(tc.tile_pool(name="dramp", bufs=1, space="DRAM"))

        # ---- PE warmup: dense dummy matmuls while DMA streams in -------
        # HAM un-throttles only after a full ~3.4us activity window of
        # sustained PE busy; dummy groups interleave with the DMA-gated
        # early production so the PE never idles long enough to re-gate.
        # The result feeds the warmup collective's input so DCE keeps it.
        scr = persist.tile([128, 64], FP16, tag="scr", name="scr")
        nc.vector.memset(scr, 0.5)
        wup = avp.tile([128, 64], FP32, tag="av", name="wup")

        def dummy_mms(n):
            for w in range(n):
                nc.tensor.matmul(wup[0:64, :], lhsT=scr, rhs=scr,
                                 start=(w == 0), stop=(w == n - 1))

        # ---- constants -------------------------------------------------
        eps_sb = persist.tile([128, 1], FP32, tag="eps", name="eps_sb")
        nc.vector.memset(eps_sb, EPS)
        ident = persist.tile([128, 128], FP32, tag="ident", name="ident")
        make_identity(nc, ident)
        ident_hf = persist.tile([128, 128], FP16, tag="identh", name="ident_hf")
        nc.vector.tensor_copy(ident_hf, ident)

        # ---- input loads, ordered by first use -------------------------
        wq_sb, wk_sb, wpv_sb = [], [], []
        for cc in range(2):
            w1 = persist.tile([128, 128], FP16, tag=f"wq{cc}", name=f"wq_sb{cc}")
            nc.sync.dma_start(w1, wq4_d[128 * cc:128 * (cc + 1), :])
            wq_sb.append(w1)
            w2 = persist.tile([128, 128], FP16, tag=f"wk{cc}", name=f"wk_sb{cc}")
            nc.sync.dma_start(w2, wk4_d[128 * cc:128 * (cc + 1), :])
            wk_sb.append(w2)
        x1_hf = [persist.tile([128, NH], FP16, tag=f"x1f_{cc}", name=f"x1_hf{cc}")
                 for cc in range(2)]
        x2_sb = [persist.tile([128, N], FP16, tag=f"x2_{cc}", name=f"x2_sb{cc}")
                 for cc in range(2)]

        # 1024-col chunks (2KB per partition line) amortize DMA packet
        # overhead; later chunks are emitted just-in-time from the main
        # loop so early bandwidth goes to the first-needed data.
        def dma_x2(chunk):
            sl = slice(1024 * chunk, 1024 * (chunk + 1))
            for cc in range(2):
                nc.sync.dma_start(x2_sb[cc][:, sl],
                                  x2b_d[128 * cc:128 * (cc + 1), sl])

        def dma_x1f(chunk):
            sl = slice(1024 * chunk, 1024 * (chunk + 1))
            for cc in range(2):
                nc.sync.dma_start(x1_hf[cc][:, sl],
                                  x1f_d[128 * cc:128 * (cc + 1), sl])

        dma_x1f(0)
        dma_x2(0)
        for cc in range(2):
            w3 = persist.tile([128, C], FP16, tag=f"wpv{cc}", name=f"wpv_sb{cc}")
            nc.sync.dma_start(w3, wpvT_d[128 * cc:128 * (cc + 1), :])
            wpv_sb.append(w3)
        dma_x2(1)

        # ---- persistent main-loop operands -----------------------------
        vt = persist.tile([128, N_JBLK, C + 1], FP16, tag="vt", name="vt")
        nc.vector.memset(vt[:, :, C:C + 1], 1.0)
        k_rep = persist.tile([128, N], FP16, tag="krep", name="k_rep")
        q_rep = persist.tile([128, NH], FP16, tag="qrep", name="q_rep")

        def emit_q(qt):
            qp = qkp.tile([128, 512], FP32, tag="qk", name=f"qp{qt}")
            for cc in range(2):
                nc.tensor.matmul(
                    qp, lhsT=wq_sb[cc],
                    rhs=x1_hf[cc][:, 512 * qt:512 * (qt + 1)],
                    start=(cc == 0), stop=(cc == 1))
            nc.vector.tensor_copy(q_rep[:, 512 * qt:512 * (qt + 1)], qp)

        def emit_k(jt):
            kp = qkp.tile([128, 512], FP32, tag="qk", name=f"kp{jt}")
            for cc in range(2):
                nc.tensor.matmul(
                    kp, lhsT=wk_sb[cc],
                    rhs=x2_sb[cc][:, 512 * jt:512 * (jt + 1)],
                    start=(cc == 0), stop=(cc == 1))
            nc.vector.tensor_copy(k_rep[:, 512 * jt:512 * (jt + 1)], kp)

        def emit_v(jblk):
            vp = qkp.tile([128, C], FP32, tag="qk", name=f"vp{jblk}")
            for cc in range(2):
                nc.tensor.matmul(
                    vp, lhsT=x2_sb[cc][:, 128 * jblk:128 * (jblk + 1)],
                    rhs=wpv_sb[cc], start=(cc == 0), stop=(cc == 1))
            nc.vector.tensor_copy(vt[:, jblk, 0:C], vp)

        def produce_for_burst(b):
            # emit k-chunk / v-blocks needed by tile-0 burst b
            if b % 2 == 0 and b // 2 >= 1:
                emit_k(b // 2)
            emit_v(2 * b)
            emit_v(2 * b + 1)

        # minimal pre-main production: q/k/v for the first two bursts,
        # with dummy-matmul groups bridging the DMA-paced gaps so HAM
        # warms once and stays warm
        dummy_mms(96)
        emit_q(0)
        dummy_mms(16)
        emit_k(0)
        dummy_mms(16)
        for b in range(2):
            emit_v(2 * b)
            dummy_mms(8)
            emit_v(2 * b + 1)
            dummy_mms(8)
        wup_sb = persist.tile([128, 4], FP32, tag="wup_sb", name="wup_sb")
        nc.vector.tensor_copy(wup_sb[0:64, :], wup[0:64, 0:4])
        nc.vector.memset(wup_sb[64:128, :], 0.0)

        # ---- warmup collective: absorb CC stream startup ----------------
        if use_collective:
            warm_in = dramp.tile([128, 4], FP32, tag="warm_i", name="warm_in")
            warm_out = dramp.tile([2, 128, 4], FP32, tag="warm_o", name="warm_out")
            nc.sync.dma_start(warm_in, wup_sb)
            nc.gpsimd.collective_compute(
                "AllGather", ALU.bypass, replica_groups=REPLICA_GROUPS,
                ins=[warm_in.opt()], outs=[warm_out.opt()])

        # ---- per-i-tile stats + AllGather state ------------------------
        proj_sb = [persist.tile([128, NH], FP16, tag=f"proj{ob}", name=f"proj_sb{ob}")
                   for ob in range(2)]
        st_sb = [persist.tile([128, 2, 6], FP32, tag=f"st{it}", name=f"st{it}")
                 for it in range(N_ITILES)]
        # gathered stats: [p, tile, rank, ob, 6]
        cc_all = persist.tile([128, N_ITILES, 2, 2, 6], FP32, tag="cc_all",
                              name="cc_all")
        st_dr = [dramp.tile([128, 12], FP32, tag=f"sti{it}", name=f"st_dr{it}")
                 for it in range(N_ITILES)]
        ag_dr = [dramp.tile([2, 128, 12], FP32, tag=f"sto{it}", name=f"ag_dr{it}")
                 for it in range(N_ITILES)]

        def emit_stats_ag(it):
            nc.sync.dma_start(st_dr[it], st_sb[it].rearrange("p o s -> p (o s)"))
            if use_collective:
                nc.gpsimd.collective_compute(
                    "AllGather", ALU.bypass, replica_groups=REPLICA_GROUPS,
                    ins=[st_dr[it].opt()], outs=[ag_dr[it].opt()])
                nc.sync.dma_start(
                    cc_all[:, it],
                    ag_dr[it].rearrange("r p c -> p r c")
                             .rearrange("p r (o s) -> p r o s", o=2))
            else:
                nc.vector.tensor_copy(cc_all[:, it, 0], st_sb[it])
                nc.vector.tensor_copy(cc_all[:, it, 1], st_sb[it])

        def emit_qk(it, jb):
            isl = slice(IT * it, IT * (it + 1))
            qk = qkp.tile([128, IT * JB_PER_BURST], FP32, tag="qk",
                          name=f"qk{it}_{jb}")
            for t in range(JB_PER_BURST):
                jblk = jb * JB_PER_BURST + t
                rt = t + 2 * (jb % 2)   # alternate row-groups between bursts
                nc.tensor.matmul(
                    qk[:, IT * t:IT * (t + 1)],
                    lhsT=k_rep[32 * rt:32 * (rt + 1),
                               JBLK * jblk:JBLK * (jblk + 1)],
                    rhs=q_rep[32 * rt:32 * (rt + 1), isl],
                    start=True, stop=True, tile_position=(32 * rt, 0))
            pt = ptp.tile([128, IT * JB_PER_BURST], FP16, tag="pt",
                          name=f"pt{it}_{jb}")
            nc.scalar.activation(out=pt, in_=qk, func=AF.Exp)
            return pt

        def emit_av(av_t, jb, pt):
            for t in range(JB_PER_BURST):
                jblk = jb * JB_PER_BURST + t
                for ib in range(4):
                    nc.tensor.matmul(
                        av_t[ib],
                        lhsT=pt[:, IT * t + 128 * ib:IT * t + 128 * (ib + 1)],
                        rhs=vt[:, jblk, :],
                        start=(jb == 0 and t == 0),
                        stop=(jb == N_JBURSTS - 1 and t == JB_PER_BURST - 1))

        def epilogue_part1(it, av_t):
            # normalize by softmax denominator, transpose [i,o] -> [o,i]
            # straight into proj_sb. transposes allocate from the qk pool
            # (tag "qk") so next-tile QK interleaves in the same PSUM slots.
            isl0 = IT * it
            for ib in range(4):
                rden = sm.tile([128, 1], FP32, tag="rden", name=f"rden{it}_{ib}")
                nc.vector.reciprocal(rden, av_t[ib][:, C:C + 1])
                avn = sm.tile([128, C], FP16, tag="avn", name=f"avn{it}_{ib}")
                nc.vector.tensor_scalar_mul(avn, in0=av_t[ib][:, 0:C], scalar1=rden)
                tp = qkp.tile([128, C], FP16, tag="qk", name=f"tp{it}_{ib}")
                nc.tensor.transpose(tp[:, 0:128], avn[:, 0:128], ident_hf)
                nc.tensor.transpose(tp[:, 128:256], avn[:, 128:256], ident_hf)
                for ob in range(2):
                    nc.vector.tensor_copy(
                        proj_sb[ob][:, isl0 + 128 * ib:isl0 + 128 * (ib + 1)],
                        tp[:, 128 * ob:128 * (ob + 1)])

        def epilogue_part2(it):
            isl = slice(IT * it, IT * (it + 1))
            for ob in range(2):
                nc.vector.bn_stats(st_sb[it][:, ob, :], proj_sb[ob][:, isl])
            emit_stats_ag(it)

        # ---- main attention loop, software-pipelined across bursts -----
        pt_hold = emit_qk(0, 0)
        pending1 = None   # (it, av_t) awaiting epilogue part 1
        pending2 = None   # it awaiting epilogue part 2
        for it in range(N_ITILES):
            av_t = [avp.tile([128, C + 1], FP32, tag="av", name=f"av{it}_{ib}")
                    for ib in range(4)]
            for jb in range(N_JBURSTS):
                last = (it == N_ITILES - 1 and jb == N_JBURSTS - 1)
                if not last:
                    nit, njb = (it, jb + 1) if jb + 1 < N_JBURSTS else (it + 1, 0)
                    pt_next = emit_qk(nit, njb)
                else:
                    pt_next = None
                emit_av(av_t, jb, pt_hold)
                pt_hold = pt_next
                if it == 0 and jb in (0, 4):
                    dma_x2(2 + jb // 4)
                if it == 0 and jb == 8:
                    dma_x1f(1)
                if it == 0 and jb <= 13:
                    produce_for_burst(jb + 2)
                if jb == 14 and it + 1 < N_ITILES:
                    emit_q(it + 1)
                if jb == 0 and pending1 is not None:
                    epilogue_part1(*pending1)
                    pending2 = pending1[0]
                    pending1 = None
                if jb == 1 and pending2 is not None:
                    epilogue_part2(pending2)
                    pending2 = None
            pending1 = (it, av_t)
        epilogue_part1(*pending1)
        epilogue_part2(N_ITILES - 1)

        # ---- final stats combine (needs all 4 gathered tiles) ----------
        mean_var = persist.tile([128, 2, 2], FP32, tag="mv2", name="mean_var")
        for ob in range(2):
            nc.vector.bn_aggr(out=mean_var[:, ob, :],
                              in_=cc_all.rearrange("p t r o s -> p (t r) o s")
                                        [:, :, ob, :])
        std2 = sm.tile([128, 2], FP32, tag="std2", name="std2")
        # sqrt's ACT table load slots in right after the last exp, hiding
        # under the collective wait
        nc.scalar.activation(out=std2, in_=mean_var[:, :, 1], func=AF.Sqrt,
                             bias=eps_sb, scale=1.0)
        rstd2 = persist.tile([128, 2], FP32, tag="rstd2", name="rstd2")
        nc.vector.reciprocal(rstd2, std2)
        # nmr = -mean * rstd  (per-partition bias for the ACT normalize)
        nmr2 = persist.tile([128, 2], FP32, tag="nmr2", name="nmr2")
        nc.vector.tensor_mul(nmr2, mean_var[:, :, 0], rstd2)
        nc.vector.tensor_scalar(out=nmr2, in0=nmr2, scalar1=-1.0, scalar2=None,
                                op0=ALU.mult)

        for ob in range(2):
            rstd = rstd2[:, ob:ob + 1]
            nmr = nmr2[:, ob:ob + 1]
            # normed = proj*rstd - mean*rstd on ACT; residual add on DVE;
            # 1024-wide chunks, 4-deep buffers so nothing stalls on WAR
            for ch in range(2):
                sl = slice(1024 * ch, 1024 * (ch + 1))
                nrm = sm4.tile([128, 1024], FP16, tag="nrm", name=f"nrm{ob}_{ch}")
                nc.scalar.activation(out=nrm, in_=proj_sb[ob][:, sl],
                                     func=AF.Identity, bias=nmr, scale=rstd)
                ot = sm4.tile([128, 1024], FP32, tag="ot", name=f"ot{ob}_{ch}")
                nc.vector.tensor_add(ot, nrm, x1_hf[ob][:, sl])
                nc.sync.dma_start(out_d[128 * ob:128 * (ob + 1), sl], ot)


_nc_cache = None


def _get_nc():
    global _nc_cache
    if _nc_cache is None:
        _nc_cache = build_nc()
    return _nc_cache


def make_in_maps(x1, x2, wq, wk, wv, wp):
    x1f = np.ascontiguousarray(x1, dtype=np.float32).reshape(B, C, N)
    x1h16 = x1f.astype(np.float16)
    x2f = np.asarray(x2, np.float32).reshape(B, C, N).astype(np.float16)
    wqT = ((np.asarray(wq, np.float32) * SCALE).T).astype(np.float16)
    wkT = (np.asarray(wk, np.float32).T).astype(np.float16)
    wq4 = np.ascontiguousarray(np.tile(wqT, (1, 4)))   # [C, 128]
    wk4 = np.ascontiguousarray(np.tile(wkT, (1, 4)))   # [C, 128]
    # fused projection: proj(v) = (wp @ wv) @ x2
    wpv = np.asarray(wp, np.float32) @ np.asarray(wv, np.float32)
    wpvT = np.ascontiguousarray(wpv.T.astype(np.float16))
    in_maps = []
    for core in range(N_CORES):
        b, h = core // 2, core % 2
        in_maps.append({
            "x1f": np.ascontiguousarray(x1h16[b, :, h * NH:(h + 1) * NH]),
            "x2b": np.ascontiguousarray(x2f[b]),
            "wq4": wq4, "wk4": wk4, "wpvT": wpvT,
        })
    return in_maps


def assemble_out(results):
    out = np.empty((B, C, N), np.float32)
    for core in range(N_CORES):
        b, h = core // 2, core % 2
        out[b, :, h * NH:(h + 1) * NH] = results[core]["out"]
    return out.reshape(B, C, 16, 16, 16)


def kernel(**inputs):
    global LAST_RESULTS
    in_maps = make_in_maps(inputs["x1"], inputs["x2"], inputs["wq"],
                           inputs["wk"], inputs["wv"], inputs["wp"])
    res = run_bass_kernel_spmd(_get_nc(), in_maps, core_ids=list(range(N_CORES)))
    LAST_RESULTS = res
    return assemble_out(res.results)


# revision 22
# speedup vs baseline: 1.0811x; 1.0354x over previous
"""Trainium2 Bass kernel for nn_CrossAttention (single-head NxN attention + proj + InstanceNorm + residual).

v4: everything from v3 (fused just-in-time k/v production, per-i-tile
stats AllGather, single-HAM-window PE stream) plus:
 - PE warmup dummy matmuls during the DMA-gated start (HAM reaches
   2.4 GHz before real work arrives)
 - host-fused projection: wpv = wp @ wv, so the V matmul directly
   produces projected values and the per-tile proj matmuls disappear
 - fp16 residual (drops the 2MB fp32 x1 load)
 - 1024-wide final normalize chunks with 4-deep buffers

Sharding: 8 cores = (batch b in 0..3) x (query-half h in 0..1).
Self-contained: hardcodes shapes B=4, C=256, D=H=W=16 (N=4096), Cr=32.
"""

import numpy as np

import concourse.bass as bass
import concourse.mybir as mybir
import concourse.tile as tile
from concourse import bacc
from concourse.bass_utils import run_bass_kernel_spmd
from concourse.masks import make_identity

B, C, N, Cr = 4, 256, 4096, 32
NH = N // 2  # query tokens per core
EPS = 1e-5
SCALE = float(Cr) ** -0.5
FP32 = mybir.dt.float32
FP16 = mybir.dt.float16

N_CORES = 8
REPLICA_GROUPS = [[0, 1], [2, 3], [4, 5], [6, 7]]

IT = 512                   # i-tile width (query columns processed together)
N_ITILES = NH // IT        # 4
JBLK = 128                 # j-block (rows per QK matmul output)
N_JBLK = N // JBLK         # 32
JB_PER_BURST = 2           # j-blocks per burst; each row-tiled QK matmul owns a full PSUM bank
N_JBURSTS = N_JBLK // JB_PER_BURST  # 16

AF = mybir.ActivationFunctionType
ALU = mybir.AluOpType

LAST_RESULTS = None  # BassKernelResults of the most recent run (for test harness)


def build_nc(use_collective=True):
    nc = bacc.Bacc("TRN2", num_devices=N_CORES, name="xattn",
                   target_bir_lowering=False)

    x1f_d = nc.dram_tensor("x1f", [C, NH], FP16, kind="ExternalInput").ap()
    x2b_d = nc.dram_tensor("x2b", [C, N], FP16, kind="ExternalInput").ap()
    wq4_d = nc.dram_tensor("wq4", [C, 128], FP16, kind="ExternalInput").ap()
    wk4_d = nc.dram_tensor("wk4", [C, 128], FP16, kind="ExternalInput").ap()
    wpvT_d = nc.dram_tensor("wpvT", [C, C], FP16, kind="ExternalInput").ap()
    out_d = nc.dram_tensor("out", [C, NH], FP32, kind="ExternalOutput").ap()

    with tile.TileContext(nc) as tc:
        build_body(tc, x1f_d, x2b_d, wq4_d, wk4_d, wpvT_d, out_d,
                   use_collective)
    nc.compile()
    return nc


def build_body(tc, x1f_d, x2b_d, wq4_d, wk4_d, wpvT_d, out_d,
               use_collective=True):
    nc = tc.nc
    from contextlib import ExitStack

    with ExitStack() as ctx:
        persist = ctx.enter_context(tc.tile_pool(name="persist", bufs=1))
        ptp = ctx.enter_context(tc.tile_pool(name="ptp", bufs=3))
        sm = ctx.enter_context(tc.tile_pool(name="sm", bufs=4))
        sm2 = ctx.enter_context(tc.tile_pool(name="sm2", bufs=2))
        sm4 = ctx.enter_context(tc.tile_pool(name="sm4", bufs=4))
        qkp = ctx.enter_context(tc.tile_pool(name="qkp", bufs=2, space="PSUM"))
        avp = ctx.enter_context(tc.tile_pool(name="avp", bufs=4, space="PSUM"))
        dramp = ctx.enter_context(tc.tile_pool(name="dramp", bufs=1, space="DRAM"))

        # ---- PE warmup: dense dummy matmuls while DMA streams in -------
        # HAM un-throttles only after a full ~3.4us activity window of
        # sustained PE busy; dummy groups interleave with the DMA-gated
        # early production so the PE never idles long enough to re-gate.
        # The result feeds the warmup collective's input so DCE keeps it.
        scr = persist.tile([128, 64], FP16, tag="scr", name="scr")
        nc.vector.memset(scr, 0.5)
        wup = avp.tile([128, 64], FP32, tag="av", name="wup")

        def dummy_mms(n):
            for w in range(n):
                nc.tensor.matmul(wup[0:64, :], lhsT=scr, rhs=scr,
                                 start=(w == 0), stop=(w == n - 1))

        # ---- constants -------------------------------------------------
        eps_sb = persist.tile([128, 1], FP32, tag="eps", name="eps_sb")
        nc.vector.memset(eps_sb, EPS)
        ident = persist.tile([128, 128], FP32, tag="ident", name="ident")
        make_identity(nc, ident)
        ident_hf = persist.tile([128, 128], FP16, tag="identh", name="ident_hf")
        nc.vector.tensor_copy(ident_hf, ident)

        # ---- input loads, ordered by first use -------------------------
        wq_sb, wk_sb, wpv_sb = [], [], []
        for cc in range(2):
            w1 = persist.tile([128, 128], FP16, tag=f"wq{cc}", name=f"wq_sb{cc}")
            nc.sync.dma_start(w1, wq4_d[128 * cc:128 * (cc + 1), :])
            wq_sb.append(w1)
            w2 = persist.tile([128, 128], FP16, tag=f"wk{cc}", name=f"wk_sb{cc}")
            nc.sync.dma_start(w2, wk4_d[128 * cc:128 * (cc + 1), :])
            wk_sb.append(w2)
        x1_hf = [persist.tile([128, NH], FP16, tag=f"x1f_{cc}", name=f"x1_hf{cc}")
                 for cc in range(2)]
        x2_sb = [persist.tile([128, N], FP16, tag=f"x2_{cc}", name=f"x2_sb{cc}")
                 for cc in range(2)]

        # 1024-col chunks (2KB per partition line) amortize DMA packet
        # overhead; later chunks are emitted just-in-time from the main
        # loop so early bandwidth goes to the first-needed data.
        def dma_x2(chunk):
            sl = slice(1024 * chunk, 1024 * (chunk + 1))
            for cc in range(2):
                nc.sync.dma_start(x2_sb[cc][:, sl],
                                  x2b_d[128 * cc:128 * (cc + 1), sl])

        def dma_x1f(chunk):
            sl = slice(1024 * chunk, 1024 * (chunk + 1))
            for cc in range(2):
                nc.sync.dma_start(x1_hf[cc][:, sl],
                                  x1f_d[128 * cc:128 * (cc + 1), sl])

        dma_x1f(0)
        dma_x2(0)
        for cc in range(2):
            w3 = persist.tile([128, C], FP16, tag=f"wpv{cc}", name=f"wpv_sb{cc}")
            nc.sync.dma_start(w3, wpvT_d[128 * cc:128 * (cc + 1), :])
            wpv_sb.append(w3)
        dma_x2(1)

        # ---- persistent main-loop operands -----------------------------
        vt = persist.tile([128, N_JBLK, C + 1], FP16, tag="vt", name="vt")
        nc.vector.memset(vt[:, :, C:C + 1], 1.0)
        k_rep = persist.tile([128, N], FP16, tag="krep", name="k_rep")
        q_rep = persist.tile([128, NH], FP16, tag="qrep", name="q_rep")

        def emit_q(qt):
            qp = qkp.tile([128, 512], FP32, tag="qk", name=f"qp{qt}")
            for cc in range(2):
                nc.tensor.matmul(
                    qp, lhsT=wq_sb[cc],
                    rhs=x1_hf[cc][:, 512 * qt:512 * (qt + 1)],
                    start=(cc == 0), stop=(cc == 1))
            nc.vector.tensor_copy(q_rep[:, 512 * qt:512 * (qt + 1)], qp)

        def emit_k(jt):
            kp = qkp.tile([128, 512], FP32, tag="qk", name=f"kp{jt}")
            for cc in range(2):
                nc.tensor.matmul(
                    kp, lhsT=wk_sb[cc],
                    rhs=x2_sb[cc][:, 512 * jt:512 * (jt + 1)],
                    start=(cc == 0), stop=(cc == 1))
            nc.vector.tensor_copy(k_rep[:, 512 * jt:512 * (jt + 1)], kp)

        def emit_v(jblk):
            vp = qkp.tile([128, C], FP32, tag="qk", name=f"vp{jblk}")
            for cc in range(2):
                nc.tensor.matmul(
                    vp, lhsT=x2_sb[cc][:, 128 * jblk:128 * (jblk + 1)],
                    rhs=wpv_sb[cc], start=(cc == 0), stop=(cc == 1))
            nc.vector.tensor_copy(vt[:, jblk, 0:C], vp)

        def produce_for_burst(b):
            # emit k-chunk / v-blocks needed by tile-0 burst b
            if b % 2 == 0 and b // 2 >= 1:
                emit_k(b // 2)
            emit_v(2 * b)
            emit_v(2 * b + 1)

        # minimal pre-main production: q/k/v for the first two bursts,
        # with dummy-matmul groups bridging the DMA-paced gaps so HAM
        # warms once and stays warm
        dummy_mms(96)
        emit_q(0)
        dummy_mms(16)
        emit_k(0)
        dummy_mms(16)
        for b in range(2):
            emit_v(2 * b)
            dummy_mms(8)
            emit_v(2 * b + 1)
            dummy_mms(8)
        wup_sb = persist.tile([128, 4], FP32, tag="wup_sb", name="wup_sb")
        nc.vector.tensor_copy(wup_sb[0:64, :], wup[0:64, 0:4])
        nc.vector.memset(wup_sb[64:128, :], 0.0)

        # ---- warmup collective: absorb CC stream startup ----------------
        if use_collective:
            warm_in = dramp.tile([128, 4], FP32, tag="warm_i", name="warm_in")
            warm_out = dramp.tile([2, 128, 4], FP32, tag="warm_o", name="warm_out")
            nc.sync.dma_start(warm_in, wup_sb)
            nc.gpsimd.collective_compute(
                "AllGather", ALU.bypass, replica_groups=REPLICA_GROUPS,
                ins=[warm_in.opt()], outs=[warm_out.opt()])

        # ---- per-i-tile stats + AllGather state ------------------------
        proj_sb = [persist.tile([128, NH], FP16, tag=f"proj{ob}", name=f"proj_sb{ob}")
                   for ob in range(2)]
        st_sb = [persist.tile([128, 2, 6], FP32, tag=f"st{it}", name=f"st{it}")
                 for it in range(N_ITILES)]
        # gathered stats: [p, tile, rank, ob, 6]
        cc_all = persist.tile([128, N_ITILES, 2, 2, 6], FP32, tag="cc_all",
                              name="cc_all")
        st_dr = [dramp.tile([128, 12], FP32, tag=f"sti{it}", name=f"st_dr{it}")
                 for it in range(N_ITILES)]
        ag_dr = [dramp.tile([2, 128, 12], FP32, tag=f"sto{it}", name=f"ag_dr{it}")
                 for it in range(N_ITILES)]

        def emit_stats_ag(it):
            nc.sync.dma_start(st_dr[it], st_sb[it].rearrange("p o s -> p (o s)"))
            if use_collective:
                nc.gpsimd.collective_compute(
                    "AllGather", ALU.bypass, replica_groups=REPLICA_GROUPS,
                    ins=[st_dr[it].opt()], outs=[ag_dr[it].opt()])
                nc.sync.dma_start(
                    cc_all[:, it],
                    ag_dr[it].rearrange("r p c -> p r c")
                             .rearrange("p r (o s) -> p r o s", o=2))
            else:
                nc.vector.tensor_copy(cc_all[:, it, 0], st_sb[it])
                nc.vector.tensor_copy(cc_all[:, it, 1], st_sb[it])

        def emit_qk(it, jb):
            isl = slice(IT * it, IT * (it + 1))
            qk = qkp.tile([128, IT * JB_PER_BURST], FP32, tag="qk",
                          name=f"qk{it}_{jb}")
            for t in range(JB_PER_BURST):
                jblk = jb * JB_PER_BURST + t
                rt = t + 2 * (jb % 2)   # alternate row-groups between bursts
                nc.tensor.matmul(
                    qk[:, IT * t:IT * (t + 1)],
                    lhsT=k_rep[32 * rt:32 * (rt + 1),
                               JBLK * jblk:JBLK * (jblk + 1)],
                    rhs=q_rep[32 * rt:32 * (rt + 1), isl],
                    start=True, stop=True, tile_position=(32 * rt, 0))
            pt = ptp.tile([128, IT * JB_PER_BURST], FP16, tag="pt",
                          name=f"pt{it}_{jb}")
            nc.scalar.activation(out=pt, in_=qk, func=AF.Exp)
            return pt

        def emit_av(av_t, jb, pt):
            for t in range(JB_PER_BURST):
                jblk = jb * JB_PER_BURST + t
                for ib in range(4):
                    nc.tensor.matmul(
                        av_t[ib],
                        lhsT=pt[:, IT * t + 128 * ib:IT * t + 128 * (ib + 1)],
                        rhs=vt[:, jblk, :],
                        start=(jb == 0 and t == 0),
                        stop=(jb == N_JBURSTS - 1 and t == JB_PER_BURST - 1))

        def epilogue_part1(it, av_t):
            # normalize by softmax denominator, transpose [i,o] -> [o,i]
            # straight into proj_sb. transposes allocate from the qk pool
            # (tag "qk") so next-tile QK interleaves in the same PSUM slots.
            isl0 = IT * it
            for ib in range(4):
                rden = sm.tile([128, 1], FP32, tag="rden", name=f"rden{it}_{ib}")
                nc.vector.reciprocal(rden, av_t[ib][:, C:C + 1])
                avn = sm.tile([128, C], FP16, tag="avn", name=f"avn{it}_{ib}")
                nc.vector.tensor_scalar_mul(avn, in0=av_t[ib][:, 0:C], scalar1=rden)
                tp = qkp.tile([128, C], FP16, tag="qk", name=f"tp{it}_{ib}")
                nc.tensor.transpose(tp[:, 0:128], avn[:, 0:128], ident_hf)
                nc.tensor.transpose(tp[:, 128:256], avn[:, 128:256], ident_hf)
                for ob in range(2):
                    nc.vector.tensor_copy(
                        proj_sb[ob][:, isl0 + 128 * ib:isl0 + 128 * (ib + 1)],
                        tp[:, 128 * ob:128 * (ob + 1)])

        def epilogue_part2(it):
            isl = slice(IT * it, IT * (it + 1))
            for ob in range(2):
                nc.vector.bn_stats(st_sb[it][:, ob, :], proj_sb[ob][:, isl])
            emit_stats_ag(it)

        # ---- main attention loop, software-pipelined across bursts -----
        pt_hold = emit_qk(0, 0)
        pending1 = None   # (it, av_t) awaiting epilogue part 1
        pending2 = None   # it awaiting epilogue part 2
        for it in range(N_ITILES):
            av_t = [avp.tile([128, C + 1], FP32, tag="av", name=f"av{it}_{ib}")
                    for ib in range(4)]
            for jb in range(N_JBURSTS):
                last = (it == N_ITILES - 1 and jb == N_JBURSTS - 1)
                if not last:
                    nit, njb = (it, jb + 1) if jb + 1 < N_JBURSTS else (it + 1, 0)
                    pt_next = emit_qk(nit, njb)
                else:
                    pt_next = None
                emit_av(av_t, jb, pt_hold)
                pt_hold = pt_next
                if it == 0 and jb in (0, 4):
                    dma_x2(2 + jb // 4)
                if it == 0 and jb == 8:
                    dma_x1f(1)
                if it == 0 and jb <= 13:
                    produce_for_burst(jb + 2)
                if jb == 14 and it + 1 < N_ITILES:
                    emit_q(it + 1)
                if jb == 0 and pending1 is not None:
                    epilogue_part1(*pending1)
                    pending2 = pending1[0]
                    pending1 = None
                if jb == 1 and pending2 is not None:
                    epilogue_part2(pending2)
                    pending2 = None
            pending1 = (it, av_t)
        epilogue_part1(*pending1)
        epilogue_part2(N_ITILES - 1)

        # ---- final stats combine (needs all 4 gathered tiles) ----------
        mean_var = persist.tile([128, 2, 2], FP32, tag="mv2", name="mean_var")
        for ob in range(2):
            nc.vector.bn_aggr(out=mean_var[:, ob, :],
                              in_=cc_all.rearrange("p t r o s -> p (t r) o s")
                                        [:, :, ob, :])
        std2 = sm.tile([128, 2], FP32, tag="std2", name="std2")
        # sqrt's ACT table load slots in right after the last exp, hiding
        # under the collective wait
        nc.scalar.activation(out=std2, in_=mean_var[:, :, 1], func=AF.Sqrt,
                             bias=eps_sb, scale=1.0)
        rstd2 = persist.tile([128, 2], FP32, tag="rstd2", name="rstd2")
        nc.vector.reciprocal(rstd2, std2)
        # nmr = -mean * rstd  (per-partition bias for the ACT normalize)
        nmr2 = persist.tile([128, 2], FP32, tag="nmr2", name="nmr2")
        nc.vector.tensor_mul(nmr2, mean_var[:, :, 0], rstd2)
        nc.vector.tensor_scalar(out=nmr2, in0=nmr2, scalar1=-1.0, scalar2=None,
                                op0=ALU.mult)

        # normed = (proj - mean)*rstd, split across ACT (Identity with
        # per-partition scale/bias) and DVE (fp16 tensor_scalar at 2x);
        # residual adds on DVE; 1024-wide chunks, 4-deep buffers
        for ch in range(2):
            for ob in range(2):
                sl = slice(1024 * ch, 1024 * (ch + 1))
                nrm = sm4.tile([128, 1024], FP16, tag="nrm", name=f"nrm{ob}_{ch}")
                if ch == 0:
                    nc.scalar.activation(out=nrm, in_=proj_sb[ob][:, sl],
                                         func=AF.Identity,
                                         bias=nmr2[:, ob:ob + 1],
                                         scale=rstd2[:, ob:ob + 1])
                else:
                    nc.vector.tensor_scalar(
                        out=nrm, in0=proj_sb[ob][:, sl],
                        scalar1=mean_var[:, ob, 0:1], scalar2=rstd2[:, ob:ob + 1],
                        op0=ALU.subtract, op1=ALU.mult)
                ot = sm4.tile([128, 1024], FP32, tag="ot", name=f"ot{ob}_{ch}")
                nc.vector.tensor_add(ot, nrm, x1_hf[ob][:, sl])
                nc.sync.dma_start(out_d[128 * ob:128 * (ob + 1), sl], ot)


_nc_cache = None


def _get_nc():
    global _nc_cache
    if _nc_cache is None:
        _nc_cache = build_nc()
    return _nc_cache


def make_in_maps(x1, x2, wq, wk, wv, wp):
    x1f = np.ascontiguousarray(x1, dtype=np.float32).reshape(B, C, N)
    x1h16 = x1f.astype(np.float16)
    x2f = np.asarray(x2, np.float32).reshape(B, C, N).astype(np.float16)
    wqT = ((np.asarray(wq, np.float32) * SCALE).T).astype(np.float16)
    wkT = (np.asarray(wk, np.float32).T).astype(np.float16)
    wq4 = np.ascontiguousarray(np.tile(wqT, (1, 4)))   # [C, 128]
    wk4 = np.ascontiguousarray(np.tile(wkT, (1, 4)))   # [C, 128]
    # fused projection: proj(v) = (wp @ wv) @ x2
    wpv = np.asarray(wp, np.float32) @ np.asarray(wv, np.float32)
    wpvT = np.ascontiguousarray(wpv.T.astype(np.float16))
    in_maps = []
    for core in range(N_CORES):
        b, h = core // 2, core % 2
        in_maps.append({
            "x1f": np.ascontiguousarray(x1h16[b, :, h * NH:(h + 1) * NH]),
            "x2b": np.ascontiguousarray(x2f[b]),
            "wq4": wq4, "wk4": wk4, "wpvT": wpvT,
        })
    return in_maps


def assemble_out(results):
    out = np.empty((B, C, N), np.float32)
    for core in range(N_CORES):
        b, h = core // 2, core % 2
        out[b, :, h * NH:(h + 1) * NH] = results[core]["out"]
    return out.reshape(B, C, 16, 16, 16)


def kernel(**inputs):
    global LAST_RESULTS
    in_maps = make_in_maps(inputs["x1"], inputs["x2"], inputs["wq"],
                           inputs["wk"], inputs["wv"], inputs["wp"])
    res = run_bass_kernel_spmd(_get_nc(), in_maps, core_ids=list(range(N_CORES)))
    LAST_RESULTS = res
    return assemble_out(res.results)
